# revision 7
# baseline (speedup 1.0000x reference)
"""AdvancedFeatureTransformer Trainium2 kernel (bf16, engine-balanced).

Data-parallel over batch: 8 cores x 512 rows, no collectives.
All activations feature-major (h^T: [feat_part, batch_free]); every matmul
operand is bf16 (1 cycle/row on the PE vs 4 for fp32), psum accumulates fp32.

Structure (driven by HW traces of earlier revisions):
  - NO gpsimd elementwise (measured 7.5us per [128,512] op); gpsimd only
    issues DMAs and one partition_broadcast.
  - Heads read each W1 psum exactly twice (ACT Square for LN stats, DVE
    tensor_scalar relu) -- no materialized pre-activation.
  - LN rstd is algebraically deferred:
       relu(Z*r + b2) = r * relu(Z + b2*sd),  sd = 1/r
    The rank-1 b2 (x) sd term is added into the W2 psum by a K=32
    selection matmul against a group-level sd tile [32t, 512b]; the final
    r scale is applied once per 32-target group on the W3 output.
  - LN stats are selection-matrix matmuls (M=32) accumulating into one
    [32, 512] psum bank per group, so sqrt/recip run once per 32 targets.
  - Head groups are software-pipelined (phase A of group g+1 is emitted
    before phase B of group g) and the stats/W3 matmuls lag one step
    behind their elementwise producers, keeping the in-order PE queue fed.
  - Trunk: residuals ride an identity matmul into the out-proj psum;
    Square reads the psum directly on ACT while the bias-add runs on DVE;
    rstd is broadcast across partitions by a K=1 ones matmul on the PE.
"""

import sys

if "/opt/trn_rl_repo" not in sys.path:
    sys.path.insert(0, "/opt/trn_rl_repo")

import numpy as np
import ml_dtypes

BF16 = ml_dtypes.bfloat16

B = 4096
NCORES = 8
BL = B // NCORES        # 512 rows per core
DIN = 512
D = 256
T = 424
L = 6
EPS = 1e-5
PAIRS = T // 2          # 212
NG = (T + 31) // 32     # 14 head groups of <=32 targets
NB8 = T // 8            # 53 W1 dma blocks of 8 targets
NPB = 27                # W2 dma blocks of 8 pairs (216 padded)

_cache = {}


def _prep(inputs):
    f32 = lambda a: np.ascontiguousarray(np.asarray(a, dtype=np.float32))
    b16 = lambda a: np.ascontiguousarray(np.asarray(a, dtype=np.float32).astype(BF16))

    x = f32(inputs["x"])
    assert np.all(np.asarray(inputs["ln_g"]) == 1.0), "ln_g != 1 unsupported"
    assert np.all(np.asarray(inputs["ln_b"]) == 0.0), "ln_b != 0 unsupported"
    assert np.all(np.asarray(inputs["tp_ln_g"]) == 1.0), "tp_ln_g != 1 unsupported"
    assert np.all(np.asarray(inputs["tp_ln_b"]) == 0.0), "tp_ln_b != 0 unsupported"

    # ---- projection ----
    Wp = f32(inputs["proj_W"]).reshape(D, DIN)        # [256, 512]
    bp = f32(inputs["proj_b"]).reshape(D)
    WpT = b16(Wp.T)                                   # [512, 256] raw
    wpm = Wp.mean(0)                                  # [512] col means
    wpmC = b16(wpm.reshape(4, 128).T)                 # [128, 4]
    bpm = float(bp.mean())

    # ---- trunk ----
    aiW = f32(inputs["attn_in_W"])
    aib = f32(inputs["attn_in_b"])
    aoW = f32(inputs["attn_out_W"])
    aob = f32(inputs["attn_out_b"])
    f1W = f32(inputs["ff_W1"])
    f1b = f32(inputs["ff_b1"])
    f2W = f32(inputs["ff_W2"])
    f2b = f32(inputs["ff_b2"])

    WvT = np.empty((L, D, D), np.float32)
    WoT = np.empty((L, D, D), np.float32)
    Wf1T = np.empty((L, D, 4 * D), np.float32)
    Wf2T = np.empty((L, 4 * D, D), np.float32)
    for i in range(L):
        WvT[i] = aiW[i, 2 * D:].T
        WoT[i] = (aoW[i] - aoW[i].mean(0, keepdims=True)).T
        Wf1T[i] = f1W[i].T
        Wf2T[i] = (f2W[i] - f2W[i].mean(0, keepdims=True)).T
    bv = aib[:, 2 * D:]
    bo = aob - aob.mean(1, keepdims=True)
    bo0p = bo[0] - bpm                                 # layer-0 merged bias
    bf1 = f1b
    bf2 = f2b - f2b.mean(1, keepdims=True)

    cW = f32(inputs["cross_in_W"])
    WcvT = b16(cW[2 * D:].T)
    bcv = f32(inputs["cross_in_b"])[2 * D:]
    WcoT = b16(f32(inputs["cross_out_W"]).T)
    bco = f32(inputs["cross_out_b"])

    # ---- trunk bias pack: [nb, 128] fp32 -> sbuf [128, nb] ----
    cols = []

    def pack(vec):
        v = f32(vec).reshape(-1, 128)
        s = len(cols)
        cols.extend(v)
        return s

    bias_idx = {
        "bp": pack(bp),
        "bv": [pack(bv[i]) for i in range(L)],
        "bo": [pack(bo0p)] + [pack(bo[i]) for i in range(1, L)],
        "bf1": [pack(bf1[i]) for i in range(L)],
        "bf2": [pack(bf2[i]) for i in range(L)],
        "bcv": pack(bcv),
        "bco": pack(bco),
    }
    TB = f32(np.stack(cols))                           # [nb, 128]

    # ---- heads ----
    W1 = f32(inputs["tp_W1"])                          # [424, 128, 256]
    b1 = f32(inputs["tp_b1"])                          # [424, 128]
    W1c = W1 - W1.mean(1, keepdims=True)
    b1c = b1 - b1.mean(1, keepdims=True)
    b1T = f32(b1c.T)                                   # [128, 424]
    # W1G[gi, k, 2*ti+c, m] = W1c[8gi+ti].T[128c+k, m]
    W1G = b16(np.transpose(
        W1c.transpose(0, 2, 1).reshape(NB8, 8, 2, 128, 128),
        (0, 3, 1, 2, 4)).reshape(NB8, 128, 16, 128))

    W2 = f32(inputs["tp_W2"])                          # [424, 64, 128]
    b2 = f32(inputs["tp_b2"])                          # [424, 64]
    W2P = W2.transpose(0, 2, 1).reshape(PAIRS, 2, 128, 64)  # [212,2,128,64]
    W2Pp = np.zeros((NPB * 8, 2, 128, 64), np.float32)
    W2Pp[:PAIRS] = W2P
    # W2G[gi, k, qi, e, m]
    W2G = b16(np.transpose(
        W2Pp.reshape(NPB, 8, 2, 128, 64), (0, 3, 1, 2, 4)
    ).reshape(NPB, 128, 1024))

    b2G = np.zeros((NG, 32, 16, 128), np.float32)
    W3 = f32(inputs["tp_W3"])                          # [424, 64]
    b3 = f32(inputs["tp_b3"])                          # [424]
    W3G = np.zeros((NG, 128, 16, 32), np.float32)
    b3B = np.zeros((32, NG), np.float32)
    for t in range(T):
        g, lt = t // 32, t % 32
        q, e = lt // 2, lt % 2
        b2G[g, lt, q, 64 * e:64 * e + 64] = b2[t]
        W3G[g, 64 * e:64 * e + 64, q, lt] = W3[t]
        b3B[lt, g] = b3[t]
    b2G = b16(b2G.reshape(NG, 32, 2048))
    W3G = b16(W3G.reshape(NG, 128, 512))
    b3B = f32(b3B)

    sel32 = b16(np.broadcast_to(np.eye(32, dtype=np.float32),
                                (128, 32, 32)).reshape(128, 1024))

    shared = {
        "WpT": WpT, "wpmC": wpmC,
        "WvT": b16(WvT), "WoT": b16(WoT),
        "Wf1T": b16(Wf1T), "Wf2T": b16(Wf2T),
        "WcvT": WcvT, "WcoT": WcoT, "TB": TB,
        "W1G": W1G, "b1T": b1T, "W2G": W2G, "b2G": b2G,
        "W3G": W3G, "b3B": b3B, "sel32": sel32,
    }
    in_maps = []
    for c in range(NCORES):
        m = dict(shared)
        m["xT"] = b16(x[c * BL:(c + 1) * BL].T)        # [512, 512]
        in_maps.append(m)
    return in_maps, TB.shape[0], bias_idx


def _build(nb, bias_idx):
    import concourse.bass as bass
    import concourse.mybir as mybir
    import concourse.tile as tile
    from concourse import bacc
    from concourse.masks import make_identity

    f32 = mybir.dt.float32
    bf = mybir.dt.bfloat16
    Alu = mybir.AluOpType
    Act = mybir.ActivationFunctionType
    ts = bass.ts

    nc = bacc.Bacc(None, target_bir_lowering=False)
    dr = lambda name, shape, dt=bf: nc.dram_tensor(name, shape, dt,
                                                   kind="ExternalInput")
    xT = dr("xT", [DIN, BL])
    WpT = dr("WpT", [DIN, D])
    wpmC = dr("wpmC", [128, 4])
    WvT = dr("WvT", [L, D, D])
    WoT = dr("WoT", [L, D, D])
    Wf1T = dr("Wf1T", [L, D, 4 * D])
    Wf2T = dr("Wf2T", [L, 4 * D, D])
    WcvT = dr("WcvT", [D, D])
    WcoT = dr("WcoT", [D, D])
    TB = dr("TB", [nb, 128], f32)
    W1G = dr("W1G", [NB8, 128, 16, 128])
    b1T = dr("b1T", [128, T], f32)
    W2G = dr("W2G", [NPB, 128, 1024])
    b2G = dr("b2G", [NG, 32, 2048])
    W3G = dr("W3G", [NG, 128, 512])
    b3B = dr("b3B", [32, NG], f32)
    sel32 = dr("sel32", [128, 1024])
    out = nc.dram_tensor("out", [BL, T], f32, kind="ExternalOutput")

    from contextlib import ExitStack

    with tile.TileContext(nc) as tc, ExitStack() as stack:
        consts = stack.enter_context(tc.tile_pool(name="consts", bufs=1))

        tb_sb = consts.tile([128, nb], f32, tag="tb")
        nc.gpsimd.dma_start(out=tb_sb, in_=TB.rearrange("n p -> p n"))
        b1_sb = consts.tile([128, T], f32, tag="b1")
        nc.gpsimd.dma_start(out=b1_sb, in_=b1T[:, :])
        b3_sb = consts.tile([32, NG], f32, tag="b3")
        nc.gpsimd.dma_start(out=b3_sb, in_=b3B[:, :])
        sel_sb = consts.tile([128, 32, 32], bf, tag="sel")
        nc.gpsimd.dma_start(out=sel_sb, in_=sel32.rearrange("p (j m) -> p j m", j=32))
        eps_col = consts.tile([128, 1], f32, tag="eps")
        nc.vector.memset(eps_col, EPS)
        ones_b = consts.tile([128, 1], bf, tag="ones")
        nc.vector.memset(ones_b, 1.0)
        ones_row = consts.tile([1, 128], bf, tag="onesr")
        nc.vector.memset(ones_row, 1.0)
        idnb = consts.tile([128, 128], bf, tag="idnb")
        make_identity(nc, idnb)
        idnf = consts.tile([128, 128], f32, tag="idnf")
        make_identity(nc, idnf)
        out_sb = [consts.tile([128, T], f32, tag=f"ob{i}", name=f"ob{i}")
                  for i in range(4)]
        hcs = [consts.tile([128, BL], bf, tag=f"hc{m}", name=f"hc{m}")
               for m in range(2)]

        def bias_col(idx, m=0):
            return tb_sb[:, idx + m:idx + m + 1]

        # ================= trunk =================
        with tc.tile_pool(name="twt", bufs=2) as twt, \
             tc.tile_pool(name="tact", bufs=3) as tact, \
             tc.tile_pool(name="hpool", bufs=2) as hpool, \
             tc.tile_pool(name="tps", bufs=2, space="PSUM") as tps, \
             tc.tile_pool(name="tpss", bufs=1, space="PSUM") as tpss, \
             tc.tile_pool(name="tcps", bufs=1, space="PSUM") as tcps, \
             tc.tile_pool(name="trbp", bufs=1, space="PSUM") as trbp:

            xs = twt.tile([128, 4, BL], bf, tag="x")
            nc.gpsimd.dma_start(out=xs, in_=xT.rearrange("(c k) b -> k c b", c=4))
            wp = twt.tile([128, 4, D], bf, tag="wp")
            nc.gpsimd.dma_start(out=wp, in_=WpT.rearrange("(c k) m -> k c m", c=4))
            wpm_sb = twt.tile([128, 4], bf, tag="wpm")
            nc.gpsimd.dma_start(out=wpm_sb, in_=wpmC[:, :])

            # c = wpm . x  (layer-0 LN mean correction)
            cps = tcps.tile([1, BL], f32, tag="cps")
            for k in range(4):
                nc.tensor.matmul(cps, wpm_sb[:, k:k + 1], xs[:, k],
                                 start=(k == 0), stop=(k == 3))
            c_sb = tact.tile([1, BL], f32, tag="c1")
            nc.scalar.activation(c_sb, cps, Act.Identity, bias=0.0, scale=1.0)
            c_bc = consts.tile([128, BL], f32, tag="cbc")
            nc.gpsimd.partition_broadcast(c_bc, c_sb, channels=128)

            # h0 = Wp.T @ x + bp   (raw, uncentered)
            hps = tps.tile([128, 2, BL], f32, tag="mm")
            for m in range(2):
                for k in range(4):
                    nc.tensor.matmul(hps[:, m, :], wp[:, k, ts(m, 128)], xs[:, k],
                                     start=(k == 0), stop=(k == 3))
            h = hpool.tile([128, 2, BL], bf, tag="h", name="h0")
            for m in range(2):
                nc.scalar.activation(h[:, m, :], hps[:, m, :], Act.Identity,
                                     bias=bias_col(bias_idx["bp"], m), scale=1.0)

            def layer_norm(yps, bcol_idx, layer0=False):
                """yps: [128, 2, BL] psum residual+proj (biasless).
                Returns normalized bf16 [128, 2, BL] tile."""
                yp = tact.tile([128, 2, BL], bf, tag="yp", name="yp")
                for m in range(2):
                    if layer0:
                        nc.vector.scalar_tensor_tensor(
                            out=yp[:, m, :], in0=yps[:, m, :],
                            scalar=bias_col(bcol_idx, m), in1=c_bc,
                            op0=Alu.add, op1=Alu.subtract)
                    else:
                        nc.vector.tensor_scalar(
                            out=yp[:, m, :], in0=yps[:, m, :],
                            scalar1=bias_col(bcol_idx, m), scalar2=None,
                            op0=Alu.add)
                ssq = tpss.tile([1, BL], f32, tag="ssq")
                for m in range(2):
                    sq = tact.tile([128, BL], bf, tag="sq")
                    if layer0:
                        nc.vector.tensor_tensor(out=sq, in0=yp[:, m, :],
                                                in1=yp[:, m, :], op=Alu.mult)
                    else:
                        nc.scalar.activation(sq, yps[:, m, :], Act.Square,
                                             bias=bias_col(bcol_idx, m),
                                             scale=1.0)
                    nc.tensor.matmul(ssq, ones_b, sq,
                                     start=(m == 0), stop=(m == 1))
                sd = tact.tile([1, BL], f32, tag="sd")
                nc.scalar.activation(sd, ssq, Act.Sqrt,
                                     bias=eps_col[0:1], scale=1.0 / D)
                r = tact.tile([1, BL], f32, tag="rr")
                nc.vector.reciprocal_approx_fast(out=r, in_=sd)
                r16 = tact.tile([1, BL], bf, tag="r16")
                nc.vector.tensor_copy(out=r16, in_=r)
                rbps = trbp.tile([128, 2, BL], f32, tag="rb")
                for m in range(2):
                    nc.tensor.matmul(rbps[:, m, :], ones_row, r16,
                                     start=True, stop=True)
                hn = hpool.tile([128, 2, BL], bf, tag="h", name="hn")
                nc.vector.tensor_tensor(out=hn, in0=yp, in1=rbps, op=Alu.mult)
                return hn

            for i in range(L):
                # attention == out_proj(v_proj(h)); residual via identity mm
                wv = twt.tile([128, 2, D], bf, tag="wv")
                nc.gpsimd.dma_start(out=wv, in_=WvT[i].rearrange(
                    "(c k) m -> k c m", c=2))
                vps = tps.tile([128, 2, BL], f32, tag="mm")
                for m in range(2):
                    for k in range(2):
                        nc.tensor.matmul(vps[:, m, :], wv[:, k, ts(m, 128)],
                                         h[:, k, :],
                                         start=(k == 0), stop=(k == 1))
                v = tact.tile([128, 2, BL], bf, tag="v", name="v")
                for m in range(2):
                    nc.scalar.activation(v[:, m, :], vps[:, m, :], Act.Identity,
                                         bias=bias_col(bias_idx["bv"][i], m),
                                         scale=1.0)
                wo = twt.tile([128, 2, D], bf, tag="wo")
                nc.gpsimd.dma_start(out=wo, in_=WoT[i].rearrange(
                    "(c k) m -> k c m", c=2))
                yps = tps.tile([128, 2, BL], f32, tag="mm")
                for m in range(2):
                    for k in range(2):
                        nc.tensor.matmul(yps[:, m, :], wo[:, k, ts(m, 128)],
                                         v[:, k, :],
                                         start=(k == 0), stop=False)
                    nc.tensor.matmul(yps[:, m, :], idnb, h[:, m, :],
                                     start=False, stop=True)
                h = layer_norm(yps, bias_idx["bo"][i], layer0=(i == 0))

                # feed-forward
                w1 = twt.tile([128, 2, 4 * D], bf, tag="wf1")
                nc.gpsimd.dma_start(out=w1, in_=Wf1T[i].rearrange(
                    "(c k) m -> k c m", c=2))
                g = tact.tile([128, 8, BL], bf, tag="g", name="g")
                for gm in range(4):
                    gps = tps.tile([128, 2, BL], f32, tag="mm")
                    for half in range(2):
                        m = 2 * gm + half
                        for k in range(2):
                            nc.tensor.matmul(gps[:, half, :],
                                             w1[:, k, ts(m, 128)], h[:, k, :],
                                             start=(k == 0), stop=(k == 1))
                        nc.scalar.activation(g[:, m, :], gps[:, half, :],
                                             Act.Gelu,
                                             bias=bias_col(bias_idx["bf1"][i], m),
                                             scale=1.0)
                w2 = twt.tile([128, 8, D], bf, tag="wf2")
                nc.gpsimd.dma_start(out=w2, in_=Wf2T[i].rearrange(
                    "(c k) m -> k c m", c=8))
                yps = tps.tile([128, 2, BL], f32, tag="mm")
                for m in range(2):
                    for k in range(8):
                        nc.tensor.matmul(yps[:, m, :], w2[:, k, ts(m, 128)],
                                         g[:, k, :],
                                         start=(k == 0), stop=False)
                    nc.tensor.matmul(yps[:, m, :], idnb, h[:, m, :],
                                     start=False, stop=True)
                h = layer_norm(yps, bias_idx["bf2"][i])

            # cross attention (residual, no LN)
            wv = twt.tile([128, 2, D], bf, tag="wv")
            nc.gpsimd.dma_start(out=wv, in_=WcvT.rearrange("(c k) m -> k c m", c=2))
            vps = tps.tile([128, 2, BL], f32, tag="mm")
            for m in range(2):
                for k in range(2):
                    nc.tensor.matmul(vps[:, m, :], wv[:, k, ts(m, 128)],
                                     h[:, k, :], start=(k == 0), stop=(k == 1))
            v = tact.tile([128, 2, BL], bf, tag="v", name="vc")
            for m in range(2):
                nc.scalar.activation(v[:, m, :], vps[:, m, :], Act.Identity,
                                     bias=bias_col(bias_idx["bcv"], m), scale=1.0)
            wo = twt.tile([128, 2, D], bf, tag="wo")
            nc.gpsimd.dma_start(out=wo, in_=WcoT.rearrange("(c k) m -> k c m", c=2))
            yps = tps.tile([128, 2, BL], f32, tag="mm")
            for m in range(2):
                for k in range(2):
                    nc.tensor.matmul(yps[:, m, :], wo[:, k, ts(m, 128)],
                                     v[:, k, :], start=(k == 0), stop=False)
                nc.tensor.matmul(yps[:, m, :], idnb, h[:, m, :],
                                 start=False, stop=True)
                nc.scalar.activation(hcs[m], yps[:, m, :], Act.Identity,
                                     bias=bias_col(bias_idx["bco"], m), scale=1.0)

        # ================= heads =================
        with tc.tile_pool(name="w1p", bufs=3) as w1p, \
             tc.tile_pool(name="w2p", bufs=2) as w2p, \
             tc.tile_pool(name="w3p", bufs=2) as w3p, \
             tc.tile_pool(name="b2p", bufs=2) as b2p, \
             tc.tile_pool(name="sqp", bufs=6) as sqp, \
             tc.tile_pool(name="Rp", bufs=72) as Rp, \
             tc.tile_pool(name="R2p", bufs=3) as R2p, \
             tc.tile_pool(name="grp", bufs=2) as grp, \
             tc.tile_pool(name="Tps", bufs=3, space="PSUM") as Tps, \
             tc.tile_pool(name="Sps", bufs=1, space="PSUM") as Sps, \
             tc.tile_pool(name="Zps", bufs=2, space="PSUM") as Zps, \
             tc.tile_pool(name="Ops", bufs=1, space="PSUM") as Ops, \
             tc.tile_pool(name="Pps", bufs=1, space="PSUM") as Pps:

            w1t_ref = [None]

            def phase_A(g):
                """T, sq, R, stats for all targets in group; stats lag one
                target behind sq so the PE queue is not blocked on ACT."""
                gs = min(32, T - 32 * g)
                ssq = Sps.tile([32, BL], f32, tag="ssq", name="ssq")
                Rlist = []
                pend = None                      # (ti, sq) awaiting stats mm
                for ti in range(gs):
                    t = 32 * g + ti
                    if t % 8 == 0:
                        w1t_ref[0] = w1p.tile([128, 16, 128], bf,
                                              tag="w1", name="w1")
                        nc.gpsimd.dma_start(out=w1t_ref[0], in_=W1G[t // 8])
                    w1t = w1t_ref[0]
                    tps_ = Tps.tile([128, BL], f32, tag="T", name="Tps")
                    for k in range(2):
                        nc.tensor.matmul(tps_, w1t[:, 2 * (t % 8) + k, :],
                                         hcs[k], start=(k == 0), stop=(k == 1))
                    sq = sqp.tile([128, BL], bf, tag="sq", name="sq")
                    nc.scalar.activation(sq, tps_, Act.Square,
                                         bias=b1_sb[:, t:t + 1], scale=1.0)
                    R = Rp.tile([128, BL], bf, tag="R", name="R")
                    nc.vector.tensor_scalar(
                        out=R, in0=tps_, scalar1=b1_sb[:, t:t + 1],
                        scalar2=0.0, op0=Alu.add, op1=Alu.max)
                    if pend is not None:
                        nc.tensor.matmul(ssq, sel_sb[:, pend[0], :], pend[1],
                                         start=(pend[0] == 0), stop=False)
                    pend = (ti, sq)
                    Rlist.append(R)
                nc.tensor.matmul(ssq, sel_sb[:, pend[0], :], pend[1],
                                 start=(pend[0] == 0), stop=True)
                sdf = grp.tile([32, BL], f32, tag="sdf", name="sdf")
                nc.scalar.activation(sdf, ssq, Act.Sqrt,
                                     bias=eps_col[0:32], scale=1.0 / 128)
                sdb = grp.tile([32, BL], bf, tag="sdb", name="sdb")
                nc.vector.tensor_copy(out=sdb, in_=sdf)
                rstd = grp.tile([32, BL], f32, tag="rst", name="rstd")
                nc.vector.reciprocal_approx_fast(out=rstd, in_=sdf)
                # prefetch phase-B weights for this group
                w3t = w3p.tile([128, 16, 32], bf, tag="w3", name="w3")
                nc.gpsimd.dma_start(out=w3t, in_=W3G[g].rearrange(
                    "p (q m) -> p q m", q=16))
                b2t = b2p.tile([32, 16, 128], bf, tag="b2", name="b2")
                nc.gpsimd.dma_start(out=b2t, in_=b2G[g].rearrange(
                    "p (q m) -> p q m", q=16))
                return (Rlist, sdb, rstd, w3t, b2t)

            w2t_ref = [None]

            def phase_B(g, state):
                """Z (+ b2 (x) sd), relu, W3 accumulation, output block."""
                Rlist, sdb, rstd, w3t, b2t = state
                gs = min(32, T - 32 * g)
                npair = gs // 2
                o3g = Ops.tile([32, BL], f32, tag="o3g", name="o3g")
                pend = None                     # (q, R2) awaiting W3 mm
                for q in range(npair):
                    p = 16 * g + q
                    if p % 8 == 0:
                        w2t_ref[0] = w2p.tile([128, 8, 2, 64], bf,
                                              tag="w2", name="w2")
                        nc.gpsimd.dma_start(
                            out=w2t_ref[0], in_=W2G[p // 8].rearrange(
                                "p (q e m) -> p q e m", q=8, e=2))
                    w2t = w2t_ref[0]
                    zps = Zps.tile([128, BL], f32, tag="z", name="zps")
                    qi = p % 8
                    # rank-1 b2 (x) sd first: full-tile start=True makes
                    # the has_written state unambiguous for partial writes.
                    nc.tensor.matmul(zps, b2t[:, q, :], sdb,
                                     start=True, stop=False)
                    nc.tensor.matmul(zps[0:64], w2t[:, qi, 0, :],
                                     Rlist[2 * q], start=False, stop=False)
                    nc.tensor.matmul(zps[64:128], w2t[:, qi, 1, :],
                                     Rlist[2 * q + 1], start=False, stop=True)
                    R2 = R2p.tile([128, BL], bf, tag="R2", name="R2")
                    if q % 2 == 0:
                        nc.scalar.activation(R2, zps, Act.Relu, bias=0.0,
                                             scale=1.0)
                    else:
                        nc.vector.tensor_scalar(out=R2, in0=zps, scalar1=0.0,
                                                scalar2=None, op0=Alu.max)
                    if pend is not None:
                        nc.tensor.matmul(o3g, w3t[:, pend[0], :], pend[1],
                                         start=(pend[0] == 0), stop=False)
                    pend = (q, R2)
                nc.tensor.matmul(o3g, w3t[:, pend[0], :], pend[1],
                                 start=(pend[0] == 0), stop=True)

                # final: scale by rstd, add b3, transpose to [batch, target]
                o3u = grp.tile([32, BL], bf, tag="o3u", name="o3u")
                nc.vector.tensor_tensor(out=o3u, in0=o3g, in1=rstd, op=Alu.mult)
                o3f = grp.tile([32, BL], f32, tag="o3f", name="o3f")
                nc.scalar.activation(o3f, o3u, Act.Identity,
                                     bias=b3_sb[0:32, g:g + 1], scale=1.0)
                for bc in range(4):
                    tp = Pps.tile([128, 32], f32, tag="tp", name="tp")
                    nc.tensor.transpose(tp[:, 0:gs],
                                        o3f[0:gs, ts(bc, 128)], idnf[0:gs, 0:gs])
                    nc.vector.tensor_copy(
                        out=out_sb[bc][:, 32 * g:32 * g + gs],
                        in_=tp[:, 0:gs])

            state = phase_A(0)
            for g in range(1, NG):
                new_state = phase_A(g)
                phase_B(g - 1, state)
                state = new_state
            phase_B(NG - 1, state)

            for bc in range(4):
                nc.gpsimd.dma_start(out=out[ts(bc, 128)], in_=out_sb[bc])

    nc.compile()
    return nc


def kernel(**inputs):
    from concourse.bass_utils import run_bass_kernel_spmd

    in_maps, nb, bias_idx = _prep(inputs)
    if "nc" not in _cache:
        _cache["nc"] = _build(nb, bias_idx)
    nc = _cache["nc"]
    import os
    res = run_bass_kernel_spmd(
        nc, in_maps, core_ids=list(range(NCORES)),
        trace=bool(int(os.environ.get("KTRACE", "0"))))
    _cache["last_result"] = res
    outs = [np.asarray(r["out"], dtype=np.float32) for r in res.results]
    return np.concatenate(outs, axis=0)


# revision 13
# speedup vs baseline: 1.0284x; 1.0284x over previous
"""AdvancedFeatureTransformer Trainium2 kernel (bf16, engine-balanced).

Data-parallel over batch: 8 cores x 512 rows, no collectives.
All activations feature-major (h^T: [feat_part, batch_free]); every matmul
operand is bf16 (1 cycle/row on the PE vs 4 for fp32), psum accumulates fp32.

Structure (driven by HW traces of earlier revisions):
  - NO gpsimd elementwise (measured 7.5us per [128,512] op); gpsimd only
    issues DMAs and one partition_broadcast.
  - Heads read each W1 psum exactly twice (ACT Square for LN stats, DVE
    tensor_scalar relu) -- no materialized pre-activation.
  - LN rstd is algebraically deferred:
       relu(Z*r + b2) = r * relu(Z + b2*sd),  sd = 1/r
    The rank-1 b2 (x) sd term is added into the W2 psum by a K=32
    selection matmul against a group-level sd tile [32t, 512b]; the final
    r scale is applied once per 32-target group on the W3 output.
  - LN stats are selection-matrix matmuls (M=32) accumulating into one
    [32, 512] psum bank per group, so sqrt/recip run once per 32 targets.
  - Head groups are software-pipelined (phase A of group g+1 is emitted
    before phase B of group g) and the stats/W3 matmuls lag one step
    behind their elementwise producers, keeping the in-order PE queue fed.
  - Trunk: residuals ride an identity matmul into the out-proj psum;
    Square reads the psum directly on ACT while the bias-add runs on DVE;
    rstd is broadcast across partitions by a K=1 ones matmul on the PE.
"""

import sys

if "/opt/trn_rl_repo" not in sys.path:
    sys.path.insert(0, "/opt/trn_rl_repo")

import numpy as np
import ml_dtypes

BF16 = ml_dtypes.bfloat16

B = 4096
NCORES = 8
BL = B // NCORES        # 512 rows per core
DIN = 512
D = 256
T = 424
L = 6
EPS = 1e-5
PAIRS = T // 2          # 212
NG = (T + 31) // 32     # 14 head groups of <=32 targets
NB8 = T // 8            # 53 W1 dma blocks of 8 targets
NPB = 27                # W2 dma blocks of 8 pairs (216 padded)

_cache = {}


def _prep(inputs):
    f32 = lambda a: np.ascontiguousarray(np.asarray(a, dtype=np.float32))
    b16 = lambda a: np.ascontiguousarray(np.asarray(a, dtype=np.float32).astype(BF16))

    x = f32(inputs["x"])
    assert np.all(np.asarray(inputs["ln_g"]) == 1.0), "ln_g != 1 unsupported"
    assert np.all(np.asarray(inputs["ln_b"]) == 0.0), "ln_b != 0 unsupported"
    assert np.all(np.asarray(inputs["tp_ln_g"]) == 1.0), "tp_ln_g != 1 unsupported"
    assert np.all(np.asarray(inputs["tp_ln_b"]) == 0.0), "tp_ln_b != 0 unsupported"

    # ---- projection ----
    Wp = f32(inputs["proj_W"]).reshape(D, DIN)        # [256, 512]
    bp = f32(inputs["proj_b"]).reshape(D)
    WpT = b16(Wp.T)                                   # [512, 256] raw
    wpm = Wp.mean(0)                                  # [512] col means
    wpmC = b16(wpm.reshape(4, 128).T)                 # [128, 4]
    bpm = float(bp.mean())

    # ---- trunk ----
    aiW = f32(inputs["attn_in_W"])
    aib = f32(inputs["attn_in_b"])
    aoW = f32(inputs["attn_out_W"])
    aob = f32(inputs["attn_out_b"])
    f1W = f32(inputs["ff_W1"])
    f1b = f32(inputs["ff_b1"])
    f2W = f32(inputs["ff_W2"])
    f2b = f32(inputs["ff_b2"])

    WvT = np.empty((L, D, D), np.float32)
    WoT = np.empty((L, D, D), np.float32)
    Wf1T = np.empty((L, D, 4 * D), np.float32)
    Wf2T = np.empty((L, 4 * D, D), np.float32)
    for i in range(L):
        WvT[i] = aiW[i, 2 * D:].T
        WoT[i] = (aoW[i] - aoW[i].mean(0, keepdims=True)).T
        Wf1T[i] = f1W[i].T
        Wf2T[i] = (f2W[i] - f2W[i].mean(0, keepdims=True)).T
    bv = aib[:, 2 * D:]
    bo = aob - aob.mean(1, keepdims=True)
    bo0p = bo[0] - bpm                                 # layer-0 merged bias
    bf1 = f1b
    bf2 = f2b - f2b.mean(1, keepdims=True)

    cW = f32(inputs["cross_in_W"])
    WcvT = b16(cW[2 * D:].T)
    bcv = f32(inputs["cross_in_b"])[2 * D:]
    WcoT = b16(f32(inputs["cross_out_W"]).T)
    bco = f32(inputs["cross_out_b"])

    # ---- trunk bias pack: [nb, 128] fp32 -> sbuf [128, nb] ----
    cols = []

    def pack(vec):
        v = f32(vec).reshape(-1, 128)
        s = len(cols)
        cols.extend(v)
        return s

    bias_idx = {
        "bp": pack(bp),
        "bv": [pack(bv[i]) for i in range(L)],
        "bo": [pack(bo0p)] + [pack(bo[i]) for i in range(1, L)],
        "bf1": [pack(bf1[i]) for i in range(L)],
        "bf2": [pack(bf2[i]) for i in range(L)],
        "bcv": pack(bcv),
        "bco": pack(bco),
    }
    TB = f32(np.stack(cols))                           # [nb, 128]

    # ---- heads ----
    W1 = f32(inputs["tp_W1"])                          # [424, 128, 256]
    b1 = f32(inputs["tp_b1"])                          # [424, 128]
    W1c = W1 - W1.mean(1, keepdims=True)
    b1c = b1 - b1.mean(1, keepdims=True)
    b1T = f32(b1c.T)                                   # [128, 424]
    # W1G[gi, k, 2*ti+c, m] = W1c[8gi+ti].T[128c+k, m]
    W1G = b16(np.transpose(
        W1c.transpose(0, 2, 1).reshape(NB8, 8, 2, 128, 128),
        (0, 3, 1, 2, 4)).reshape(NB8, 128, 16, 128))

    W2 = f32(inputs["tp_W2"])                          # [424, 64, 128]
    b2 = f32(inputs["tp_b2"])                          # [424, 64]
    W2P = W2.transpose(0, 2, 1).reshape(PAIRS, 2, 128, 64)  # [212,2,128,64]
    W2Pp = np.zeros((NPB * 8, 2, 128, 64), np.float32)
    W2Pp[:PAIRS] = W2P
    # W2G[gi, k, qi, e, m]
    W2G = b16(np.transpose(
        W2Pp.reshape(NPB, 8, 2, 128, 64), (0, 3, 1, 2, 4)
    ).reshape(NPB, 128, 1024))

    b2G = np.zeros((NG, 32, 16, 128), np.float32)
    W3 = f32(inputs["tp_W3"])                          # [424, 64]
    b3 = f32(inputs["tp_b3"])                          # [424]
    W3G = np.zeros((NG, 128, 16, 32), np.float32)
    b3B = np.zeros((32, NG), np.float32)
    for t in range(T):
        g, lt = t // 32, t % 32
        q, e = lt // 2, lt % 2
        b2G[g, lt, q, 64 * e:64 * e + 64] = b2[t]
        W3G[g, 64 * e:64 * e + 64, q, lt] = W3[t]
        b3B[lt, g] = b3[t]
    b2G = b16(b2G.reshape(NG, 32, 2048))
    W3G = b16(W3G.reshape(NG, 128, 512))
    b3B = f32(b3B)

    sel32 = b16(np.broadcast_to(np.eye(32, dtype=np.float32),
                                (128, 32, 32)).reshape(128, 1024))

    shared = {
        "WpT": WpT, "wpmC": wpmC,
        "WvT": b16(WvT), "WoT": b16(WoT),
        "Wf1T": b16(Wf1T), "Wf2T": b16(Wf2T),
        "WcvT": WcvT, "WcoT": WcoT, "TB": TB,
        "W1G": W1G, "b1T": b1T, "W2G": W2G, "b2G": b2G,
        "W3G": W3G, "b3B": b3B, "sel32": sel32,
    }
    in_maps = []
    for c in range(NCORES):
        m = dict(shared)
        m["xT"] = b16(x[c * BL:(c + 1) * BL].T)        # [512, 512]
        in_maps.append(m)
    return in_maps, TB.shape[0], bias_idx


def _build(nb, bias_idx):
    import concourse.bass as bass
    import concourse.mybir as mybir
    import concourse.tile as tile
    from concourse import bacc
    from concourse.masks import make_identity

    f32 = mybir.dt.float32
    bf = mybir.dt.bfloat16
    Alu = mybir.AluOpType
    Act = mybir.ActivationFunctionType
    ts = bass.ts

    nc = bacc.Bacc(None, target_bir_lowering=False)
    dr = lambda name, shape, dt=bf: nc.dram_tensor(name, shape, dt,
                                                   kind="ExternalInput")
    xT = dr("xT", [DIN, BL])
    WpT = dr("WpT", [DIN, D])
    wpmC = dr("wpmC", [128, 4])
    WvT = dr("WvT", [L, D, D])
    WoT = dr("WoT", [L, D, D])
    Wf1T = dr("Wf1T", [L, D, 4 * D])
    Wf2T = dr("Wf2T", [L, 4 * D, D])
    WcvT = dr("WcvT", [D, D])
    WcoT = dr("WcoT", [D, D])
    TB = dr("TB", [nb, 128], f32)
    W1G = dr("W1G", [NB8, 128, 16, 128])
    b1T = dr("b1T", [128, T], f32)
    W2G = dr("W2G", [NPB, 128, 1024])
    b2G = dr("b2G", [NG, 32, 2048])
    W3G = dr("W3G", [NG, 128, 512])
    b3B = dr("b3B", [32, NG], f32)
    sel32 = dr("sel32", [128, 1024])
    out = nc.dram_tensor("out", [BL, T], f32, kind="ExternalOutput")

    from contextlib import ExitStack

    with tile.TileContext(nc) as tc, ExitStack() as stack:
        consts = stack.enter_context(tc.tile_pool(name="consts", bufs=1))

        tb_sb = consts.tile([128, nb], f32, tag="tb")
        nc.gpsimd.dma_start(out=tb_sb, in_=TB.rearrange("n p -> p n"))
        b1_sb = consts.tile([128, T], f32, tag="b1")
        nc.gpsimd.dma_start(out=b1_sb, in_=b1T[:, :])
        b3_sb = consts.tile([32, NG], f32, tag="b3")
        nc.gpsimd.dma_start(out=b3_sb, in_=b3B[:, :])
        sel_sb = consts.tile([128, 32, 32], bf, tag="sel")
        nc.gpsimd.dma_start(out=sel_sb, in_=sel32.rearrange("p (j m) -> p j m", j=32))
        eps_col = consts.tile([128, 1], f32, tag="eps")
        nc.vector.memset(eps_col, EPS)
        ones_b = consts.tile([128, 1], bf, tag="ones")
        nc.vector.memset(ones_b, 1.0)
        ones_row = consts.tile([1, 128], bf, tag="onesr")
        nc.vector.memset(ones_row, 1.0)
        idnb = consts.tile([128, 128], bf, tag="idnb")
        make_identity(nc, idnb)
        idnf = consts.tile([128, 128], f32, tag="idnf")
        make_identity(nc, idnf)
        # bf16 transposed output staging: [batch 128, b-block 4, 432 (=NG*32 padded targets)]
        out16 = consts.tile([128, 4, 32 * NG + 16], bf, tag="o16")
        out_sb = [consts.tile([128, T], f32, tag=f"ob{i}", name=f"ob{i}")
                  for i in range(4)]
        hcs = [consts.tile([128, BL], bf, tag=f"hc{m}", name=f"hc{m}")
               for m in range(2)]

        def bias_col(idx, m=0):
            return tb_sb[:, idx + m:idx + m + 1]

        # ================= trunk =================
        with tc.tile_pool(name="twt", bufs=2) as twt, \
             tc.tile_pool(name="tact", bufs=3) as tact, \
             tc.tile_pool(name="hpool", bufs=2) as hpool, \
             tc.tile_pool(name="tps", bufs=2, space="PSUM") as tps, \
             tc.tile_pool(name="tpss", bufs=1, space="PSUM") as tpss, \
             tc.tile_pool(name="tcps", bufs=1, space="PSUM") as tcps, \
             tc.tile_pool(name="trbp", bufs=1, space="PSUM") as trbp:

            xs = twt.tile([128, 4, BL], bf, tag="x")
            nc.gpsimd.dma_start(out=xs, in_=xT.rearrange("(c k) b -> k c b", c=4))
            wp = twt.tile([128, 4, D], bf, tag="wp")
            nc.gpsimd.dma_start(out=wp, in_=WpT.rearrange("(c k) m -> k c m", c=4))
            wpm_sb = twt.tile([128, 4], bf, tag="wpm")
            nc.gpsimd.dma_start(out=wpm_sb, in_=wpmC[:, :])

            # c = wpm . x  (layer-0 LN mean correction)
            cps = tcps.tile([1, BL], f32, tag="cps")
            for k in range(4):
                nc.tensor.matmul(cps, wpm_sb[:, k:k + 1], xs[:, k],
                                 start=(k == 0), stop=(k == 3))
            c_sb = tact.tile([1, BL], f32, tag="c1")
            nc.scalar.activation(c_sb, cps, Act.Identity, bias=0.0, scale=1.0)
            c_bc = consts.tile([128, BL], f32, tag="cbc")
            nc.gpsimd.partition_broadcast(c_bc, c_sb, channels=128)

            # h0 = Wp.T @ x + bp   (raw, uncentered)
            hps = tps.tile([128, 2, BL], f32, tag="mm")
            for m in range(2):
                for k in range(4):
                    nc.tensor.matmul(hps[:, m, :], wp[:, k, ts(m, 128)], xs[:, k],
                                     start=(k == 0), stop=(k == 3))
            h = hpool.tile([128, 2, BL], bf, tag="h", name="h0")
            for m in range(2):
                nc.scalar.activation(h[:, m, :], hps[:, m, :], Act.Identity,
                                     bias=bias_col(bias_idx["bp"], m), scale=1.0)

            def layer_norm(yps, bcol_idx, layer0=False):
                """yps: [128, 2, BL] psum residual+proj (biasless).
                Returns normalized bf16 [128, 2, BL] tile."""
                yp = tact.tile([128, 2, BL], bf, tag="yp", name="yp")
                for m in range(2):
                    if layer0:
                        nc.vector.scalar_tensor_tensor(
                            out=yp[:, m, :], in0=yps[:, m, :],
                            scalar=bias_col(bcol_idx, m), in1=c_bc,
                            op0=Alu.add, op1=Alu.subtract)
                    else:
                        nc.vector.tensor_scalar(
                            out=yp[:, m, :], in0=yps[:, m, :],
                            scalar1=bias_col(bcol_idx, m), scalar2=None,
                            op0=Alu.add)
                ssq = tpss.tile([1, BL], f32, tag="ssq")
                for m in range(2):
                    sq = tact.tile([128, BL], bf, tag="sq")
                    if layer0:
                        nc.vector.tensor_tensor(out=sq, in0=yp[:, m, :],
                                                in1=yp[:, m, :], op=Alu.mult)
                    else:
                        nc.scalar.activation(sq, yps[:, m, :], Act.Square,
                                             bias=bias_col(bcol_idx, m),
                                             scale=1.0)
                    nc.tensor.matmul(ssq, ones_b, sq,
                                     start=(m == 0), stop=(m == 1))
                sd = tact.tile([1, BL], f32, tag="sd")
                nc.scalar.activation(sd, ssq, Act.Sqrt,
                                     bias=eps_col[0:1], scale=1.0 / D)
                r = tact.tile([1, BL], f32, tag="rr")
                nc.vector.reciprocal_approx_fast(out=r, in_=sd)
                r16 = tact.tile([1, BL], bf, tag="r16")
                nc.vector.tensor_copy(out=r16, in_=r)
                rbps = trbp.tile([128, 2, BL], f32, tag="rb")
                for m in range(2):
                    nc.tensor.matmul(rbps[:, m, :], ones_row, r16,
                                     start=True, stop=True)
                hn = hpool.tile([128, 2, BL], bf, tag="h", name="hn")
                nc.vector.tensor_tensor(out=hn, in0=yp, in1=rbps, op=Alu.mult)
                return hn

            for i in range(L):
                # attention == out_proj(v_proj(h)); residual via identity mm
                wv = twt.tile([128, 2, D], bf, tag="wv")
                nc.gpsimd.dma_start(out=wv, in_=WvT[i].rearrange(
                    "(c k) m -> k c m", c=2))
                vps = tps.tile([128, 2, BL], f32, tag="mm")
                for m in range(2):
                    for k in range(2):
                        nc.tensor.matmul(vps[:, m, :], wv[:, k, ts(m, 128)],
                                         h[:, k, :],
                                         start=(k == 0), stop=(k == 1))
                v = tact.tile([128, 2, BL], bf, tag="v", name="v")
                for m in range(2):
                    nc.scalar.activation(v[:, m, :], vps[:, m, :], Act.Identity,
                                         bias=bias_col(bias_idx["bv"][i], m),
                                         scale=1.0)
                wo = twt.tile([128, 2, D], bf, tag="wo")
                nc.gpsimd.dma_start(out=wo, in_=WoT[i].rearrange(
                    "(c k) m -> k c m", c=2))
                yps = tps.tile([128, 2, BL], f32, tag="mm")
                for m in range(2):
                    for k in range(2):
                        nc.tensor.matmul(yps[:, m, :], wo[:, k, ts(m, 128)],
                                         v[:, k, :],
                                         start=(k == 0), stop=False)
                    nc.tensor.matmul(yps[:, m, :], idnb, h[:, m, :],
                                     start=False, stop=True)
                h = layer_norm(yps, bias_idx["bo"][i], layer0=(i == 0))

                # feed-forward
                w1 = twt.tile([128, 2, 4 * D], bf, tag="wf1")
                nc.gpsimd.dma_start(out=w1, in_=Wf1T[i].rearrange(
                    "(c k) m -> k c m", c=2))
                g = tact.tile([128, 8, BL], bf, tag="g", name="g")
                for gm in range(4):
                    gps = tps.tile([128, 2, BL], f32, tag="mm")
                    for half in range(2):
                        m = 2 * gm + half
                        for k in range(2):
                            nc.tensor.matmul(gps[:, half, :],
                                             w1[:, k, ts(m, 128)], h[:, k, :],
                                             start=(k == 0), stop=(k == 1))
                        nc.scalar.activation(g[:, m, :], gps[:, half, :],
                                             Act.Gelu,
                                             bias=bias_col(bias_idx["bf1"][i], m),
                                             scale=1.0)
                w2 = twt.tile([128, 8, D], bf, tag="wf2")
                nc.gpsimd.dma_start(out=w2, in_=Wf2T[i].rearrange(
                    "(c k) m -> k c m", c=8))
                yps = tps.tile([128, 2, BL], f32, tag="mm")
                for m in range(2):
                    for k in range(8):
                        nc.tensor.matmul(yps[:, m, :], w2[:, k, ts(m, 128)],
                                         g[:, k, :],
                                         start=(k == 0), stop=False)
                    nc.tensor.matmul(yps[:, m, :], idnb, h[:, m, :],
                                     start=False, stop=True)
                h = layer_norm(yps, bias_idx["bf2"][i])

            # cross attention (residual, no LN)
            wv = twt.tile([128, 2, D], bf, tag="wv")
            nc.gpsimd.dma_start(out=wv, in_=WcvT.rearrange("(c k) m -> k c m", c=2))
            vps = tps.tile([128, 2, BL], f32, tag="mm")
            for m in range(2):
                for k in range(2):
                    nc.tensor.matmul(vps[:, m, :], wv[:, k, ts(m, 128)],
                                     h[:, k, :], start=(k == 0), stop=(k == 1))
            v = tact.tile([128, 2, BL], bf, tag="v", name="vc")
            for m in range(2):
                nc.scalar.activation(v[:, m, :], vps[:, m, :], Act.Identity,
                                     bias=bias_col(bias_idx["bcv"], m), scale=1.0)
            wo = twt.tile([128, 2, D], bf, tag="wo")
            nc.gpsimd.dma_start(out=wo, in_=WcoT.rearrange("(c k) m -> k c m", c=2))
            yps = tps.tile([128, 2, BL], f32, tag="mm")
            for m in range(2):
                for k in range(2):
                    nc.tensor.matmul(yps[:, m, :], wo[:, k, ts(m, 128)],
                                     v[:, k, :], start=(k == 0), stop=False)
                nc.tensor.matmul(yps[:, m, :], idnb, h[:, m, :],
                                 start=False, stop=True)
                nc.scalar.activation(hcs[m], yps[:, m, :], Act.Identity,
                                     bias=bias_col(bias_idx["bco"], m), scale=1.0)

        # ================= heads =================
        with tc.tile_pool(name="w1p", bufs=4) as w1p, \
             tc.tile_pool(name="w2p", bufs=2) as w2p, \
             tc.tile_pool(name="w3p", bufs=2) as w3p, \
             tc.tile_pool(name="b2p", bufs=2) as b2p, \
             tc.tile_pool(name="sqp", bufs=6) as sqp, \
             tc.tile_pool(name="Rp", bufs=72) as Rp, \
             tc.tile_pool(name="R2p", bufs=4) as R2p, \
             tc.tile_pool(name="grp", bufs=2) as grp, \
             tc.tile_pool(name="Tps", bufs=4, space="PSUM") as Tps, \
             tc.tile_pool(name="Sps", bufs=1, space="PSUM") as Sps, \
             tc.tile_pool(name="Zps", bufs=2, space="PSUM") as Zps, \
             tc.tile_pool(name="Ops", bufs=1, space="PSUM") as Ops:

            w1t_ref = [None]

            def phase_A(g):
                """T, sq, R, stats for all targets in group; stats lag one
                target behind sq so the PE queue is not blocked on ACT."""
                gs = min(32, T - 32 * g)
                ssq = Sps.tile([32, BL], f32, tag="ssq", name="ssq")
                Rlist = []
                pend = []                        # (ti, sq) awaiting stats mm
                for ti in range(gs):
                    t = 32 * g + ti
                    if t % 8 == 0:
                        w1t_ref[0] = w1p.tile([128, 16, 128], bf,
                                              tag="w1", name="w1")
                        nc.gpsimd.dma_start(out=w1t_ref[0], in_=W1G[t // 8])
                    w1t = w1t_ref[0]
                    tps_ = Tps.tile([128, BL], f32, tag="T", name="Tps")
                    for k in range(2):
                        nc.tensor.matmul(tps_, w1t[:, 2 * (t % 8) + k, :],
                                         hcs[k], start=(k == 0), stop=(k == 1))
                    sq = sqp.tile([128, BL], bf, tag="sq", name="sq")
                    nc.scalar.activation(sq, tps_, Act.Square,
                                         bias=b1_sb[:, t:t + 1], scale=1.0)
                    R = Rp.tile([128, BL], bf, tag="R", name="R")
                    nc.vector.tensor_scalar(
                        out=R, in0=tps_, scalar1=b1_sb[:, t:t + 1],
                        scalar2=0.0, op0=Alu.add, op1=Alu.max)
                    pend.append((ti, sq))
                    if len(pend) > 2:            # stats lag 2 targets
                        j, sqj = pend.pop(0)
                        nc.tensor.matmul(ssq, sel_sb[:, j, :], sqj,
                                         start=(j == 0), stop=False)
                    Rlist.append(R)
                for n, (j, sqj) in enumerate(pend):
                    nc.tensor.matmul(ssq, sel_sb[:, j, :], sqj,
                                     start=(j == 0), stop=(n == len(pend) - 1))
                sdf = grp.tile([32, BL], f32, tag="sdf", name="sdf")
                nc.scalar.activation(sdf, ssq, Act.Sqrt,
                                     bias=eps_col[0:32], scale=1.0 / 128)
                sdb = grp.tile([32, BL], bf, tag="sdb", name="sdb")
                nc.vector.tensor_copy(out=sdb, in_=sdf)
                rstd = grp.tile([32, BL], f32, tag="rst", name="rstd")
                nc.vector.reciprocal_approx_fast(out=rstd, in_=sdf)
                # prefetch phase-B weights for this group
                w3t = w3p.tile([128, 16, 32], bf, tag="w3", name="w3")
                nc.gpsimd.dma_start(out=w3t, in_=W3G[g].rearrange(
                    "p (q m) -> p q m", q=16))
                b2t = b2p.tile([32, 16, 128], bf, tag="b2", name="b2")
                nc.gpsimd.dma_start(out=b2t, in_=b2G[g].rearrange(
                    "p (q m) -> p q m", q=16))
                return (Rlist, sdb, rstd, w3t, b2t)

            w2t_ref = [None]

            def phase_B(g, state):
                """Z (+ b2 (x) sd), relu, W3 accumulation, output block."""
                Rlist, sdb, rstd, w3t, b2t = state
                gs = min(32, T - 32 * g)
                npair = gs // 2
                o3g = Ops.tile([32, BL], f32, tag="o3g", name="o3g")
                pend = []                       # (q, R2) awaiting W3 mm
                for q in range(npair):
                    p = 16 * g + q
                    if p % 8 == 0:
                        w2t_ref[0] = w2p.tile([128, 8, 2, 64], bf,
                                              tag="w2", name="w2")
                        nc.gpsimd.dma_start(
                            out=w2t_ref[0], in_=W2G[p // 8].rearrange(
                                "p (q e m) -> p q e m", q=8, e=2))
                    w2t = w2t_ref[0]
                    zps = Zps.tile([128, BL], f32, tag="z", name="zps")
                    qi = p % 8
                    # rank-1 b2 (x) sd first: full-tile start=True makes
                    # the has_written state unambiguous for partial writes.
                    nc.tensor.matmul(zps, b2t[:, q, :], sdb,
                                     start=True, stop=False)
                    nc.tensor.matmul(zps[0:64], w2t[:, qi, 0, :],
                                     Rlist[2 * q], start=False, stop=False)
                    nc.tensor.matmul(zps[64:128], w2t[:, qi, 1, :],
                                     Rlist[2 * q + 1], start=False, stop=True)
                    R2 = R2p.tile([128, BL], bf, tag="R2", name="R2")
                    if q % 2 == 0:
                        nc.scalar.activation(R2, zps, Act.Relu, bias=0.0,
                                             scale=1.0)
                    else:
                        nc.vector.tensor_scalar(out=R2, in0=zps, scalar1=0.0,
                                                scalar2=None, op0=Alu.max)
                    pend.append((q, R2))
                    if len(pend) > 2:           # W3 lag 2 pairs
                        j, R2j = pend.pop(0)
                        nc.tensor.matmul(o3g, w3t[:, j, :], R2j,
                                         start=(j == 0), stop=False)
                for n, (j, R2j) in enumerate(pend):
                    nc.tensor.matmul(o3g, w3t[:, j, :], R2j,
                                     start=(j == 0), stop=(n == len(pend) - 1))

                # final: scale by rstd, add b3, xbar-transpose to [batch, tgt]
                o3u = grp.tile([32, BL], bf, tag="o3u", name="o3u")
                nc.vector.tensor_tensor(out=o3u, in0=o3g, in1=rstd, op=Alu.mult)
                o3f = grp.tile([32, BL], bf, tag="o3f", name="o3f")
                nc.scalar.activation(o3f, o3u, Act.Identity,
                                     bias=b3_sb[0:32, g:g + 1], scale=1.0)
                rows = gs if gs % 16 == 0 else 16
                nc.sync.dma_start_transpose(
                    out16[:, :, 32 * g:32 * g + rows], o3f[0:rows, :])

            state = phase_A(0)
            for g in range(1, NG):
                new_state = phase_A(g)
                phase_B(g - 1, state)
                state = new_state
            phase_B(NG - 1, state)

            for bc in range(4):
                nc.vector.tensor_copy(out=out_sb[bc], in_=out16[:, bc, 0:T])
                nc.gpsimd.dma_start(out=out[ts(bc, 128)], in_=out_sb[bc])

    nc.compile()
    return nc


def kernel(**inputs):
    from concourse.bass_utils import run_bass_kernel_spmd

    in_maps, nb, bias_idx = _prep(inputs)
    if "nc" not in _cache:
        _cache["nc"] = _build(nb, bias_idx)
    nc = _cache["nc"]
    import os
    res = run_bass_kernel_spmd(
        nc, in_maps, core_ids=list(range(NCORES)),
        trace=bool(int(os.environ.get("KTRACE", "0"))))
    _cache["last_result"] = res
    outs = [np.asarray(r["out"], dtype=np.float32) for r in res.results]
    return np.concatenate(outs, axis=0)


# revision 26
# speedup vs baseline: 1.0332x; 1.0047x over previous
"""AdvancedFeatureTransformer Trainium2 kernel (bf16, engine-balanced).

Data-parallel over batch: 8 cores x 512 rows, no collectives.
All activations feature-major (h^T: [feat_part, batch_free]); every matmul
operand is bf16 (1 cycle/row on the PE vs 4 for fp32), psum accumulates fp32.

Structure (driven by HW traces of earlier revisions):
  - NO gpsimd elementwise (measured 7.5us per [128,512] op); gpsimd only
    issues DMAs and one partition_broadcast.
  - Heads read each W1 psum exactly twice (ACT Square for LN stats, DVE
    tensor_scalar relu) -- no materialized pre-activation.
  - LN rstd is algebraically deferred:
       relu(Z*r + b2) = r * relu(Z + b2*sd),  sd = 1/r
    The rank-1 b2 (x) sd term is added into the W2 psum by a K=32
    selection matmul against a group-level sd tile [32t, 512b]; the final
    r scale is applied once per 32-target group on the W3 output.
  - LN stats are selection-matrix matmuls (M=32) accumulating into one
    [32, 512] psum bank per group, so sqrt/recip run once per 32 targets.
  - Head groups are software-pipelined (phase A of group g+1 is emitted
    before phase B of group g) and the stats/W3 matmuls lag one step
    behind their elementwise producers, keeping the in-order PE queue fed.
  - Trunk: residuals ride an identity matmul into the out-proj psum;
    Square reads the psum directly on ACT while the bias-add runs on DVE;
    rstd is broadcast across partitions by a K=1 ones matmul on the PE.
"""

import sys

if "/opt/trn_rl_repo" not in sys.path:
    sys.path.insert(0, "/opt/trn_rl_repo")

import numpy as np
import ml_dtypes

BF16 = ml_dtypes.bfloat16

B = 4096
NCORES = 8
BL = B // NCORES        # 512 rows per core
DIN = 512
D = 256
T = 424
L = 6
EPS = 1e-5
PAIRS = T // 2          # 212
NG = (T + 31) // 32     # 14 head groups of <=32 targets
NB8 = T // 8            # 53 W1 dma blocks of 8 targets
NPB = 27                # W2 dma blocks of 8 pairs (216 padded)

_cache = {}


def _prep(inputs):
    f32 = lambda a: np.ascontiguousarray(np.asarray(a, dtype=np.float32))
    b16 = lambda a: np.ascontiguousarray(np.asarray(a, dtype=np.float32).astype(BF16))

    x = f32(inputs["x"])
    assert np.all(np.asarray(inputs["ln_g"]) == 1.0), "ln_g != 1 unsupported"
    assert np.all(np.asarray(inputs["ln_b"]) == 0.0), "ln_b != 0 unsupported"
    assert np.all(np.asarray(inputs["tp_ln_g"]) == 1.0), "tp_ln_g != 1 unsupported"
    assert np.all(np.asarray(inputs["tp_ln_b"]) == 0.0), "tp_ln_b != 0 unsupported"

    # ---- projection ----
    Wp = f32(inputs["proj_W"]).reshape(D, DIN)        # [256, 512]
    bp = f32(inputs["proj_b"]).reshape(D)
    WpT = b16(Wp.T)                                   # [512, 256] raw
    wpm = Wp.mean(0)                                  # [512] col means
    wpmC = b16(wpm.reshape(4, 128).T)                 # [128, 4]
    bpm = float(bp.mean())

    # ---- trunk ----
    aiW = f32(inputs["attn_in_W"])
    aib = f32(inputs["attn_in_b"])
    aoW = f32(inputs["attn_out_W"])
    aob = f32(inputs["attn_out_b"])
    f1W = f32(inputs["ff_W1"])
    f1b = f32(inputs["ff_b1"])
    f2W = f32(inputs["ff_W2"])
    f2b = f32(inputs["ff_b2"])

    WvT = np.empty((L, D, D), np.float32)
    WoT = np.empty((L, D, D), np.float32)
    Wf1T = np.empty((L, D, 4 * D), np.float32)
    Wf2T = np.empty((L, 4 * D, D), np.float32)
    for i in range(L):
        WvT[i] = aiW[i, 2 * D:].T
        WoT[i] = (aoW[i] - aoW[i].mean(0, keepdims=True)).T
        Wf1T[i] = f1W[i].T
        Wf2T[i] = (f2W[i] - f2W[i].mean(0, keepdims=True)).T
    bv = aib[:, 2 * D:]
    bo = aob - aob.mean(1, keepdims=True)
    bo0p = bo[0] - bpm                                 # layer-0 merged bias
    bf1 = f1b
    bf2 = f2b - f2b.mean(1, keepdims=True)

    cW = f32(inputs["cross_in_W"])
    WcvT = b16(cW[2 * D:].T)
    bcv = f32(inputs["cross_in_b"])[2 * D:]
    WcoT = b16(f32(inputs["cross_out_W"]).T)
    bco = f32(inputs["cross_out_b"])

    # ---- trunk bias pack: [nb, 128] fp32 -> sbuf [128, nb] ----
    cols = []

    def pack(vec):
        v = f32(vec).reshape(-1, 128)
        s = len(cols)
        cols.extend(v)
        return s

    bias_idx = {
        "bp": pack(bp),
        "bv": [pack(bv[i]) for i in range(L)],
        "bo": [pack(bo0p)] + [pack(bo[i]) for i in range(1, L)],
        "bf1": [pack(bf1[i]) for i in range(L)],
        "bf2": [pack(bf2[i]) for i in range(L)],
        "bcv": pack(bcv),
        "bco": pack(bco),
    }
    TB = f32(np.stack(cols))                           # [nb, 128]

    # ---- heads ----
    W1 = f32(inputs["tp_W1"])                          # [424, 128, 256]
    b1 = f32(inputs["tp_b1"])                          # [424, 128]
    W1c = W1 - W1.mean(1, keepdims=True)
    b1c = b1 - b1.mean(1, keepdims=True)
    b1T = f32(b1c.T)                                   # [128, 424]
    # W1G[gi, k, 2*ti+c, m] = W1c[8gi+ti].T[128c+k, m]
    W1G = b16(np.transpose(
        W1c.transpose(0, 2, 1).reshape(NB8, 8, 2, 128, 128),
        (0, 3, 1, 2, 4)).reshape(NB8, 128, 16, 128))

    W2 = f32(inputs["tp_W2"])                          # [424, 64, 128]
    b2 = f32(inputs["tp_b2"])                          # [424, 64]
    W2P = W2.transpose(0, 2, 1).reshape(PAIRS, 2, 128, 64)  # [212,2,128,64]
    W2Pp = np.zeros((NPB * 8, 2, 128, 64), np.float32)
    W2Pp[:PAIRS] = W2P
    # W2G[gi, k, qi, e, m]
    W2G = b16(np.transpose(
        W2Pp.reshape(NPB, 8, 2, 128, 64), (0, 3, 1, 2, 4)
    ).reshape(NPB, 128, 1024))

    b2G = np.zeros((NG, 32, 16, 128), np.float32)
    W3 = f32(inputs["tp_W3"])                          # [424, 64]
    b3 = f32(inputs["tp_b3"])                          # [424]
    W3G = np.zeros((NG, 128, 16, 32), np.float32)
    b3B = np.zeros((32, NG), np.float32)
    for t in range(T):
        g, lt = t // 32, t % 32
        q, e = lt // 2, lt % 2
        b2G[g, lt, q, 64 * e:64 * e + 64] = b2[t]
        W3G[g, 64 * e:64 * e + 64, q, lt] = W3[t]
        b3B[lt, g] = b3[t]
    b2G = b16(b2G.reshape(NG, 32, 2048))
    W3G = b16(W3G.reshape(NG, 128, 512))
    b3B = f32(b3B)

    sel32 = b16(np.broadcast_to(np.eye(32, dtype=np.float32),
                                (128, 32, 32)).reshape(128, 1024))

    shared = {
        "WpT": WpT, "wpmC": wpmC,
        "WvT": b16(WvT), "WoT": b16(WoT),
        "Wf1T": b16(Wf1T), "Wf2T": b16(Wf2T),
        "WcvT": WcvT, "WcoT": WcoT, "TB": TB,
        "W1G": W1G, "b1T": b1T, "W2G": W2G, "b2G": b2G,
        "W3G": W3G, "b3B": b3B, "sel32": sel32,
    }
    in_maps = []
    for c in range(NCORES):
        m = dict(shared)
        m["xT"] = b16(x[c * BL:(c + 1) * BL].T)        # [512, 512]
        in_maps.append(m)
    return in_maps, TB.shape[0], bias_idx


def _build(nb, bias_idx):
    import concourse.bass as bass
    import concourse.mybir as mybir
    import concourse.tile as tile
    from concourse import bacc
    from concourse.masks import make_identity

    f32 = mybir.dt.float32
    bf = mybir.dt.bfloat16
    Alu = mybir.AluOpType
    Act = mybir.ActivationFunctionType
    ts = bass.ts

    nc = bacc.Bacc(None, target_bir_lowering=False)
    dr = lambda name, shape, dt=bf: nc.dram_tensor(name, shape, dt,
                                                   kind="ExternalInput")
    xT = dr("xT", [DIN, BL])
    WpT = dr("WpT", [DIN, D])
    wpmC = dr("wpmC", [128, 4])
    WvT = dr("WvT", [L, D, D])
    WoT = dr("WoT", [L, D, D])
    Wf1T = dr("Wf1T", [L, D, 4 * D])
    Wf2T = dr("Wf2T", [L, 4 * D, D])
    WcvT = dr("WcvT", [D, D])
    WcoT = dr("WcoT", [D, D])
    TB = dr("TB", [nb, 128], f32)
    W1G = dr("W1G", [NB8, 128, 16, 128])
    b1T = dr("b1T", [128, T], f32)
    W2G = dr("W2G", [NPB, 128, 1024])
    b2G = dr("b2G", [NG, 32, 2048])
    W3G = dr("W3G", [NG, 128, 512])
    b3B = dr("b3B", [32, NG], f32)
    sel32 = dr("sel32", [128, 1024])
    out = nc.dram_tensor("out", [BL, T], f32, kind="ExternalOutput")

    from contextlib import ExitStack

    with tile.TileContext(nc) as tc, ExitStack() as stack:
        consts = stack.enter_context(tc.tile_pool(name="consts", bufs=1))

        tb_sb = consts.tile([128, nb], f32, tag="tb")
        nc.gpsimd.dma_start(out=tb_sb, in_=TB.rearrange("n p -> p n"))
        b1_sb = consts.tile([128, T], f32, tag="b1")
        nc.gpsimd.dma_start(out=b1_sb, in_=b1T[:, :])
        b3_sb = consts.tile([32, NG], f32, tag="b3")
        nc.gpsimd.dma_start(out=b3_sb, in_=b3B[:, :])
        sel_sb = consts.tile([128, 32, 32], bf, tag="sel")
        nc.gpsimd.dma_start(out=sel_sb, in_=sel32.rearrange("p (j m) -> p j m", j=32))
        eps_col = consts.tile([128, 1], f32, tag="eps")
        nc.vector.memset(eps_col, EPS)
        ones_b = consts.tile([128, 1], bf, tag="ones")
        nc.vector.memset(ones_b, 1.0)
        ones_row = consts.tile([1, 128], bf, tag="onesr")
        nc.vector.memset(ones_row, 1.0)
        idnb = consts.tile([128, 128], bf, tag="idnb")
        make_identity(nc, idnb)
        idnf = consts.tile([128, 128], f32, tag="idnf")
        make_identity(nc, idnf)
        # bf16 transposed output staging: [batch 128, b-block 4, 432 (=NG*32 padded targets)]
        out16 = consts.tile([128, 4, 32 * NG + 16], bf, tag="o16")
        out_sb = [consts.tile([128, T], f32, tag=f"ob{i}", name=f"ob{i}")
                  for i in range(4)]
        hcs = [consts.tile([128, BL], bf, tag=f"hc{m}", name=f"hc{m}")
               for m in range(2)]

        def bias_col(idx, m=0):
            return tb_sb[:, idx + m:idx + m + 1]

        # ================= trunk =================
        with tc.tile_pool(name="twt", bufs=2) as twt, \
             tc.tile_pool(name="tact", bufs=3) as tact, \
             tc.tile_pool(name="hpool", bufs=2) as hpool, \
             tc.tile_pool(name="tps", bufs=2, space="PSUM") as tps, \
             tc.tile_pool(name="tpss", bufs=1, space="PSUM") as tpss, \
             tc.tile_pool(name="tcps", bufs=1, space="PSUM") as tcps, \
             tc.tile_pool(name="trbp", bufs=1, space="PSUM") as trbp:

            xs = twt.tile([128, 4, BL], bf, tag="x")
            nc.gpsimd.dma_start(out=xs, in_=xT.rearrange("(c k) b -> k c b", c=4))
            wp = twt.tile([128, 4, D], bf, tag="wp")
            nc.gpsimd.dma_start(out=wp, in_=WpT.rearrange("(c k) m -> k c m", c=4))
            wpm_sb = twt.tile([128, 4], bf, tag="wpm")
            nc.gpsimd.dma_start(out=wpm_sb, in_=wpmC[:, :])

            # c = wpm . x  (layer-0 LN mean correction)
            cps = tcps.tile([1, BL], f32, tag="cps")
            for k in range(4):
                nc.tensor.matmul(cps, wpm_sb[:, k:k + 1], xs[:, k],
                                 start=(k == 0), stop=(k == 3))
            c_sb = tact.tile([1, BL], f32, tag="c1")
            nc.scalar.activation(c_sb, cps, Act.Identity, bias=0.0, scale=1.0)
            c_bc = consts.tile([128, BL], f32, tag="cbc")
            nc.gpsimd.partition_broadcast(c_bc, c_sb, channels=128)

            # h0 = Wp.T @ x + bp   (raw, uncentered)
            hps = tps.tile([128, 2, BL], f32, tag="mm")
            for k in range(4):
                for m in range(2):
                    nc.tensor.matmul(hps[:, m, :], wp[:, k, ts(m, 128)], xs[:, k],
                                     start=(k == 0), stop=(k == 3))
            h = hpool.tile([128, 2, BL], bf, tag="h", name="h0")
            for m in range(2):
                nc.scalar.activation(h[:, m, :], hps[:, m, :], Act.Identity,
                                     bias=bias_col(bias_idx["bp"], m), scale=1.0)

            def layer_norm(yps, bcol_idx, layer0=False):
                """yps: [128, 2, BL] psum residual+proj (biasless).
                Returns normalized bf16 [128, 2, BL] tile."""
                yp = tact.tile([128, 2, BL], bf, tag="yp", name="yp")
                for m in range(2):
                    if layer0:
                        nc.vector.scalar_tensor_tensor(
                            out=yp[:, m, :], in0=yps[:, m, :],
                            scalar=bias_col(bcol_idx, m), in1=c_bc,
                            op0=Alu.add, op1=Alu.subtract)
                    else:
                        nc.vector.tensor_scalar(
                            out=yp[:, m, :], in0=yps[:, m, :],
                            scalar1=bias_col(bcol_idx, m), scalar2=None,
                            op0=Alu.add)
                ssq = tpss.tile([1, BL], f32, tag="ssq")
                for m in range(2):
                    sq = tact.tile([128, BL], bf, tag="sq")
                    if layer0:
                        nc.vector.tensor_tensor(out=sq, in0=yp[:, m, :],
                                                in1=yp[:, m, :], op=Alu.mult)
                    else:
                        nc.scalar.activation(sq, yps[:, m, :], Act.Square,
                                             bias=bias_col(bcol_idx, m),
                                             scale=1.0)
                    nc.tensor.matmul(ssq, ones_b, sq,
                                     start=(m == 0), stop=(m == 1))
                sd = tact.tile([1, BL], f32, tag="sd")
                nc.scalar.activation(sd, ssq, Act.Sqrt,
                                     bias=eps_col[0:1], scale=1.0 / D)
                r = tact.tile([1, BL], f32, tag="rr")
                nc.vector.reciprocal_approx_fast(out=r, in_=sd)
                r16 = tact.tile([1, BL], bf, tag="r16")
                nc.vector.tensor_copy(out=r16, in_=r)
                rbps = trbp.tile([128, 2, BL], f32, tag="rb")
                for m in range(2):
                    nc.tensor.matmul(rbps[:, m, :], ones_row, r16,
                                     start=True, stop=True)
                hn = hpool.tile([128, 2, BL], bf, tag="h", name="hn")
                nc.vector.tensor_tensor(out=hn, in0=yp, in1=rbps, op=Alu.mult)
                return hn

            for i in range(L):
                # attention == out_proj(v_proj(h)); residual via identity mm
                wv = twt.tile([128, 2, D], bf, tag="wv")
                nc.gpsimd.dma_start(out=wv, in_=WvT[i].rearrange(
                    "(c k) m -> k c m", c=2))
                vps = tps.tile([128, 2, BL], f32, tag="mm")
                for k in range(2):
                    for m in range(2):
                        nc.tensor.matmul(vps[:, m, :], wv[:, k, ts(m, 128)],
                                         h[:, k, :],
                                         start=(k == 0), stop=(k == 1))
                v = tact.tile([128, 2, BL], bf, tag="v", name="v")
                for m in range(2):
                    nc.scalar.activation(v[:, m, :], vps[:, m, :], Act.Identity,
                                         bias=bias_col(bias_idx["bv"][i], m),
                                         scale=1.0)
                wo = twt.tile([128, 2, D], bf, tag="wo")
                nc.gpsimd.dma_start(out=wo, in_=WoT[i].rearrange(
                    "(c k) m -> k c m", c=2))
                yps = tps.tile([128, 2, BL], f32, tag="mm")
                for k in range(2):
                    for m in range(2):
                        nc.tensor.matmul(yps[:, m, :], wo[:, k, ts(m, 128)],
                                         v[:, k, :],
                                         start=(k == 0), stop=False)
                for m in range(2):
                    nc.tensor.matmul(yps[:, m, :], idnb, h[:, m, :],
                                     start=False, stop=True)
                h = layer_norm(yps, bias_idx["bo"][i], layer0=(i == 0))

                # feed-forward
                w1 = twt.tile([128, 2, 4 * D], bf, tag="wf1")
                nc.gpsimd.dma_start(out=w1, in_=Wf1T[i].rearrange(
                    "(c k) m -> k c m", c=2))
                g = tact.tile([128, 8, BL], bf, tag="g", name="g")
                for gm in range(4):
                    gps = tps.tile([128, 2, BL], f32, tag="mm")
                    for k in range(2):
                        for half in range(2):
                            m = 2 * gm + half
                            nc.tensor.matmul(gps[:, half, :],
                                             w1[:, k, ts(m, 128)], h[:, k, :],
                                             start=(k == 0), stop=(k == 1))
                    for half in range(2):
                        m = 2 * gm + half
                        nc.scalar.activation(g[:, m, :], gps[:, half, :],
                                             Act.Gelu,
                                             bias=bias_col(bias_idx["bf1"][i], m),
                                             scale=1.0)
                w2 = twt.tile([128, 8, D], bf, tag="wf2")
                nc.gpsimd.dma_start(out=w2, in_=Wf2T[i].rearrange(
                    "(c k) m -> k c m", c=8))
                yps = tps.tile([128, 2, BL], f32, tag="mm")
                for k in range(8):
                    for m in range(2):
                        nc.tensor.matmul(yps[:, m, :], w2[:, k, ts(m, 128)],
                                         g[:, k, :],
                                         start=(k == 0), stop=False)
                for m in range(2):
                    nc.tensor.matmul(yps[:, m, :], idnb, h[:, m, :],
                                     start=False, stop=True)
                h = layer_norm(yps, bias_idx["bf2"][i])

            # cross attention (residual, no LN)
            wv = twt.tile([128, 2, D], bf, tag="wv")
            nc.gpsimd.dma_start(out=wv, in_=WcvT.rearrange("(c k) m -> k c m", c=2))
            vps = tps.tile([128, 2, BL], f32, tag="mm")
            for k in range(2):
                for m in range(2):
                    nc.tensor.matmul(vps[:, m, :], wv[:, k, ts(m, 128)],
                                     h[:, k, :], start=(k == 0), stop=(k == 1))
            v = tact.tile([128, 2, BL], bf, tag="v", name="vc")
            for m in range(2):
                nc.scalar.activation(v[:, m, :], vps[:, m, :], Act.Identity,
                                     bias=bias_col(bias_idx["bcv"], m), scale=1.0)
            wo = twt.tile([128, 2, D], bf, tag="wo")
            nc.gpsimd.dma_start(out=wo, in_=WcoT.rearrange("(c k) m -> k c m", c=2))
            yps = tps.tile([128, 2, BL], f32, tag="mm")
            for k in range(2):
                for m in range(2):
                    nc.tensor.matmul(yps[:, m, :], wo[:, k, ts(m, 128)],
                                     v[:, k, :], start=(k == 0), stop=False)
            for m in range(2):
                nc.tensor.matmul(yps[:, m, :], idnb, h[:, m, :],
                                 start=False, stop=True)
                nc.scalar.activation(hcs[m], yps[:, m, :], Act.Identity,
                                     bias=bias_col(bias_idx["bco"], m), scale=1.0)

        # ================= heads =================
        with tc.tile_pool(name="w1p", bufs=4) as w1p, \
             tc.tile_pool(name="w2p", bufs=2) as w2p, \
             tc.tile_pool(name="w3p", bufs=2) as w3p, \
             tc.tile_pool(name="b2p", bufs=2) as b2p, \
             tc.tile_pool(name="sqp", bufs=6) as sqp, \
             tc.tile_pool(name="Rp", bufs=72) as Rp, \
             tc.tile_pool(name="R2p", bufs=4) as R2p, \
             tc.tile_pool(name="grp", bufs=2) as grp, \
             tc.tile_pool(name="Tps", bufs=4, space="PSUM") as Tps, \
             tc.tile_pool(name="Sps", bufs=1, space="PSUM") as Sps, \
             tc.tile_pool(name="Zps", bufs=2, space="PSUM") as Zps, \
             tc.tile_pool(name="Ops", bufs=1, space="PSUM") as Ops:

            w1t_ref = [None]

            def phase_A(g):
                """T, sq, R, stats for all targets in group; stats lag one
                target behind sq so the PE queue is not blocked on ACT."""
                gs = min(32, T - 32 * g)
                ssq = Sps.tile([32, BL], f32, tag="ssq", name="ssq")
                Rlist = []
                pend = []                        # (ti, sq) awaiting stats mm
                for base in range(0, gs, 2):
                    t0 = 32 * g + base
                    if t0 % 8 == 0:
                        w1t_ref[0] = w1p.tile([128, 16, 128], bf,
                                              tag="w1", name="w1")
                        nc.gpsimd.dma_start(out=w1t_ref[0], in_=W1G[t0 // 8])
                    w1t = w1t_ref[0]
                    tpair = [Tps.tile([128, BL], f32, tag="T", name="Tps")
                             for _ in range(2)]
                    # chunk-major across the two targets: consecutive matmuls
                    # hit different psum banks so fills overlap drains; one
                    # lagged stats matmul is spread between each chunk pair
                    for k in range(2):
                        for e in range(2):
                            t = t0 + e
                            nc.tensor.matmul(
                                tpair[e], w1t[:, 2 * (t % 8) + k, :], hcs[k],
                                start=(k == 0), stop=(k == 1))
                        if len(pend) > 2:
                            j, sqj = pend.pop(0)
                            nc.tensor.matmul(ssq, sel_sb[:, j, :], sqj,
                                             start=(j == 0), stop=False)
                    for e in range(2):
                        t = t0 + e
                        sq = sqp.tile([128, BL], bf, tag="sq", name="sq")
                        nc.scalar.activation(sq, tpair[e], Act.Square,
                                             bias=b1_sb[:, t:t + 1], scale=1.0)
                        R = Rp.tile([128, BL], bf, tag="R", name="R")
                        nc.vector.tensor_scalar(
                            out=R, in0=tpair[e], scalar1=b1_sb[:, t:t + 1],
                            scalar2=0.0, op0=Alu.add, op1=Alu.max)
                        pend.append((base + e, sq))
                        Rlist.append(R)
                for n, (j, sqj) in enumerate(pend):
                    nc.tensor.matmul(ssq, sel_sb[:, j, :], sqj,
                                     start=(j == 0), stop=(n == len(pend) - 1))
                sdf = grp.tile([32, BL], f32, tag="sdf", name="sdf")
                nc.scalar.activation(sdf, ssq, Act.Sqrt,
                                     bias=eps_col[0:32], scale=1.0 / 128)
                sdb = grp.tile([32, BL], bf, tag="sdb", name="sdb")
                nc.vector.tensor_copy(out=sdb, in_=sdf)
                rstd = grp.tile([32, BL], f32, tag="rst", name="rstd")
                nc.vector.reciprocal_approx_fast(out=rstd, in_=sdf)
                # prefetch phase-B weights for this group
                w3t = w3p.tile([128, 16, 32], bf, tag="w3", name="w3")
                nc.gpsimd.dma_start(out=w3t, in_=W3G[g].rearrange(
                    "p (q m) -> p q m", q=16))
                b2t = b2p.tile([32, 16, 128], bf, tag="b2", name="b2")
                nc.gpsimd.dma_start(out=b2t, in_=b2G[g].rearrange(
                    "p (q m) -> p q m", q=16))
                return (Rlist, sdb, rstd, w3t, b2t)

            w2t_ref = [None]

            def phase_B(g, state):
                """Z (+ b2 (x) sd), relu, W3 accumulation, output block."""
                Rlist, sdb, rstd, w3t, b2t = state
                gs = min(32, T - 32 * g)
                npair = gs // 2
                o3g = Ops.tile([32, BL], f32, tag="o3g", name="o3g")
                pend = []                       # (q, R2) awaiting W3 mm
                for qb in range(0, npair, 2):
                    qs = [qb] + ([qb + 1] if qb + 1 < npair else [])
                    if (16 * g + qb) % 8 == 0:
                        w2t_ref[0] = w2p.tile([128, 8, 2, 64], bf,
                                              tag="w2", name="w2")
                        nc.gpsimd.dma_start(
                            out=w2t_ref[0], in_=W2G[(16 * g + qb) // 8].rearrange(
                                "p (q e m) -> p q e m", q=8, e=2))
                    w2t = w2t_ref[0]
                    # two pairs interleaved: consecutive matmuls alternate
                    # between the two zps banks so fills overlap drains
                    zl = [Zps.tile([128, BL], f32, tag="z", name="zps")
                          for _ in qs]
                    def flush_w3():
                        if len(pend) > 2:
                            j, R2j = pend.pop(0)
                            nc.tensor.matmul(o3g, w3t[:, j, :], R2j,
                                             start=(j == 0), stop=False)

                    for q, z in zip(qs, zl):
                        nc.tensor.matmul(z, b2t[:, q, :], sdb,
                                         start=True, stop=False)
                    flush_w3()
                    for q, z in zip(qs, zl):
                        nc.tensor.matmul(z[0:64], w2t[:, (16 * g + q) % 8, 0, :],
                                         Rlist[2 * q], start=False, stop=False)
                    flush_w3()
                    for q, z in zip(qs, zl):
                        nc.tensor.matmul(z[64:128], w2t[:, (16 * g + q) % 8, 1, :],
                                         Rlist[2 * q + 1], start=False, stop=True)
                    for q, z in zip(qs, zl):
                        R2 = R2p.tile([128, BL], bf, tag="R2", name="R2")
                        if q % 2 == 0:
                            nc.scalar.activation(R2, z, Act.Relu, bias=0.0,
                                                 scale=1.0)
                        else:
                            nc.vector.tensor_scalar(out=R2, in0=z, scalar1=0.0,
                                                    scalar2=None, op0=Alu.max)
                        pend.append((q, R2))
                for n, (j, R2j) in enumerate(pend):
                    nc.tensor.matmul(o3g, w3t[:, j, :], R2j,
                                     start=(j == 0), stop=(n == len(pend) - 1))

                # final: scale by rstd, add b3, xbar-transpose to [batch, tgt]
                o3u = grp.tile([32, BL], bf, tag="o3u", name="o3u")
                nc.vector.tensor_tensor(out=o3u, in0=o3g, in1=rstd, op=Alu.mult)
                o3f = grp.tile([32, BL], bf, tag="o3f", name="o3f")
                nc.scalar.activation(o3f, o3u, Act.Identity,
                                     bias=b3_sb[0:32, g:g + 1], scale=1.0)
                rows = gs if gs % 16 == 0 else 16
                nc.sync.dma_start_transpose(
                    out16[:, :, 32 * g:32 * g + rows], o3f[0:rows, :])

            state = phase_A(0)
            for g in range(1, NG):
                new_state = phase_A(g)
                phase_B(g - 1, state)
                state = new_state
            phase_B(NG - 1, state)

            for bc in range(4):
                nc.vector.tensor_copy(out=out_sb[bc], in_=out16[:, bc, 0:T])
                nc.gpsimd.dma_start(out=out[ts(bc, 128)], in_=out_sb[bc])

    nc.compile()
    return nc


def kernel(**inputs):
    from concourse.bass_utils import run_bass_kernel_spmd

    in_maps, nb, bias_idx = _prep(inputs)
    if "nc" not in _cache:
        _cache["nc"] = _build(nb, bias_idx)
    nc = _cache["nc"]
    import os
    res = run_bass_kernel_spmd(
        nc, in_maps, core_ids=list(range(NCORES)),
        trace=bool(int(os.environ.get("KTRACE", "0"))))
    _cache["last_result"] = res
    outs = [np.asarray(r["out"], dtype=np.float32) for r in res.results]
    return np.concatenate(outs, axis=0)
